# revision 12
# baseline (speedup 1.0000x reference)
"""ChannelSelfAttn Trainium2 kernel.

Reference computation (per sample b, x_b: [C=64, T=4000]):
    q = w1*x + b1, k = w2*x + b2 broadcast over F=16 feature maps
    e[i,j] = sum_{f,t} q[f,i,t]*k[f,j,t]
           = A*G[i,j] + B1*s_i + B2*s_j + C0*T
      where G = x_b @ x_b.T, s = rowsum(x_b),
            A = w1.w2, B1 = w1.b2, B2 = b1.w2, C0 = b1.b2
    e <- (e - min_j e)/(max_j e - min_j e + 1e-8)   # row terms B1*s_i, C0*T cancel
    e <- softmax_j(e)
    out = gamma * (e @ x_b) + x_b

So only f = A*G + B2*s_j survives the normalize; everything reduces to a
64x64 gram matrix + row-softmax + a second small matmul per sample.

Sharding: data-parallel over batch. 32 samples / 8 cores = 4 samples/core,
processed as 2 pairs; each pair stacks 2 samples' channels into the 128
SBUF partitions.
"""

import numpy as np

import concourse.bacc as bacc
import concourse.bass as bass
import concourse.mybir as mybir
import concourse.tile as tile
from concourse.bass_utils import run_bass_kernel_spmd

FP32 = mybir.dt.float32
AF = mybir.ActivationFunctionType
ALU = mybir.AluOpType
AX = mybir.AxisListType

B, C, T = 32, 64, 4000
N_CORES = 8
SPC = B // N_CORES          # samples per core = 4
PAIRS = SPC // 2            # 2
TPAD = 4096                 # T padded to 32 chunks of 128
NCHUNK = TPAD // 128        # 32
NATT = TPAD // 512          # 8 attention N-chunks


def build_program(A: float, B2: float, gamma: float) -> bass.Bass:
    nc = bacc.Bacc(None)
    x_h = nc.declare_dram_parameter("x", [SPC * C, T], FP32, isOutput=False)
    id_h = nc.declare_dram_parameter("ident", [128, 128], FP32, isOutput=False)
    out_h = nc.declare_dram_parameter("out", [SPC * C, T], FP32, isOutput=True)

    with tile.TileContext(nc) as tc:
        with (
            tc.tile_pool(name="xio", bufs=2) as p_x,
            tc.tile_pool(name="xT", bufs=2) as p_xT,
            tc.tile_pool(name="outb", bufs=2) as p_out,
            tc.tile_pool(name="small", bufs=2) as p_small,
            tc.tile_pool(name="const", bufs=1) as p_const,
            tc.tile_pool(name="pt", bufs=2, space="PSUM") as p_pt,
            tc.tile_pool(name="pg", bufs=2, space="PSUM") as p_pg,
            tc.tile_pool(name="pa", bufs=2, space="PSUM") as p_pa,
            tc.tile_pool(name="ps", bufs=2, space="PSUM") as p_ps,
        ):
            ident = p_const.tile([128, 128], FP32)
            nc.sync.dma_start(ident[:], id_h[:, :])
            ones = p_const.tile([1, 128], FP32)
            nc.vector.memset(ones[:], 1.0)

            for p in range(PAIRS):
                rows = slice(p * 128, (p + 1) * 128)

                # ---- load x pair [128, 4000], zero-pad t to 4096
                x_stack = p_x.tile([128, TPAD], FP32)
                for i in range(4):
                    c0, c1 = i * 1000, (i + 1) * 1000
                    nc.sync.dma_start(x_stack[:, c0:c1], x_h[rows, c0:c1])
                nc.vector.memset(x_stack[:, T:TPAD], 0.0)

                # ---- transpose to xT [t, c] chunks via PE
                xT = p_xT.tile([128, TPAD], FP32)
                for q in range(NCHUNK // 4):
                    pt = p_pt.tile([128, 512], FP32, tag="pt")
                    for j in range(4):
                        k = 4 * q + j
                        nc.tensor.transpose(
                            pt[:, j * 128:(j + 1) * 128],
                            x_stack[:, k * 128:(k + 1) * 128],
                            ident[:, :],
                        )
                    dst = xT[:, q * 512:(q + 1) * 512]
                    if q % 2 == 0:
                        nc.vector.tensor_copy(dst, pt[:])
                    else:
                        nc.scalar.copy(dst, pt[:])

                # ---- gram: pg = sum_k xT_k.T @ xT_k  -> [[G_A, .], [., G_B]]
                pg = p_pg.tile([128, 128], FP32, tag="pg")
                for k in range(NCHUNK):
                    blkk = xT[:, k * 128:(k + 1) * 128]
                    nc.tensor.matmul(
                        pg[:], lhsT=blkk, rhs=blkk,
                        start=(k == 0), stop=(k == NCHUNK - 1),
                    )

                # ---- s = rowsum(x), then broadcast B2*s as a row
                s_col = p_small.tile([128, 1], FP32, tag="scol")
                nc.vector.reduce_sum(s_col[:], x_stack[:], axis=AX.X)
                ps_row = p_ps.tile([1, 128], FP32, tag="ps")
                nc.tensor.transpose(ps_row[:], s_col[:], ident[:, :])
                srow_b = p_small.tile([1, 128], FP32, tag="srowb")
                nc.scalar.mul(srow_b[:], ps_row[:], B2)
                # broadcast row down all partitions: psb[m, j] = B2*s_j
                psb = p_ps.tile([128, 128], FP32, tag="ps")
                nc.tensor.matmul(psb[:], lhsT=ones[:], rhs=srow_b[:], start=True, stop=True)
                sbs = p_small.tile([128, C], FP32, tag="sbs")
                nc.scalar.copy(sbs[0:64, :], psb[0:64, 0:64])
                nc.scalar.copy(sbs[64:128, :], psb[64:128, 64:128])

                # ---- f = A*G + B2*s_j (diag blocks only), stacked [128, 64]
                fs = p_small.tile([128, C], FP32, tag="fs")
                nc.vector.scalar_tensor_tensor(
                    fs[0:64, :], pg[0:64, 0:64], A, sbs[0:64, :],
                    op0=ALU.mult, op1=ALU.add,
                )
                nc.vector.scalar_tensor_tensor(
                    fs[64:128, :], pg[64:128, 64:128], A, sbs[64:128, :],
                    op0=ALU.mult, op1=ALU.add,
                )

                # ---- row minmax-normalize + exp (+ rowsum for softmax denom)
                mx = p_small.tile([128, 1], FP32, tag="mx")
                nc.vector.reduce_max(mx[:], fs[:], axis=AX.X)
                mn = p_small.tile([128, 1], FP32, tag="mn")
                nc.vector.tensor_reduce(mn[:], fs[:], axis=AX.X, op=ALU.min)
                dd = p_small.tile([128, 1], FP32, tag="dd")
                nc.vector.scalar_tensor_tensor(
                    dd[:], mx[:], 1e-8, mn[:], op0=ALU.add, op1=ALU.subtract,
                )
                rr = p_small.tile([128, 1], FP32, tag="rr")
                nc.vector.reciprocal(rr[:], dd[:])
                nb = p_small.tile([128, 1], FP32, tag="nb")
                nc.vector.scalar_tensor_tensor(
                    nb[:], mn[:], -1.0, rr[:], op0=ALU.mult, op1=ALU.mult,
                )
                # exp() written into diagonal blocks of a zeroed [128,128] tile
                # so ONE transpose yields the block-diag lhsT for the attn matmul
                gw = p_small.tile([128, 128], FP32, tag="gw")
                nc.vector.memset(gw[:], 0.0)
                se = p_small.tile([128, 1], FP32, tag="se")
                nc.scalar.activation(
                    gw[0:64, 0:64], fs[0:64, :], AF.Exp,
                    bias=nb[0:64], scale=rr[0:64], accum_out=se[0:64],
                )
                nc.scalar.activation(
                    gw[64:128, 64:128], fs[64:128, :], AF.Exp,
                    bias=nb[64:128], scale=rr[64:128], accum_out=se[64:128],
                )
                rs = p_small.tile([128, 1], FP32, tag="rs")
                nc.vector.reciprocal(rs[:], se[:])
                wsc = p_small.tile([128, 1], FP32, tag="wsc")
                nc.vector.tensor_scalar_mul(wsc[:], rs[:], gamma)

                # ---- block-diag transposed weights for attn matmul
                pb = p_ps.tile([128, 128], FP32, tag="ps")
                nc.tensor.transpose(pb[:], gw[:], ident[:, :])
                blk = p_small.tile([128, 128], FP32, tag="blk")
                nc.vector.tensor_copy(blk[:], pb[:])

                # ---- attn = blk.T @ x (unnormalized), out = wsc*attn + x
                ob = p_out.tile([128, TPAD], FP32)
                for n in range(NATT):
                    pa = p_pa.tile([128, 512], FP32, tag="pa")
                    xch = x_stack[:, n * 512:(n + 1) * 512]
                    nc.tensor.matmul(pa[:], lhsT=blk[:], rhs=xch, start=True, stop=True)
                    nc.vector.scalar_tensor_tensor(
                        ob[:, n * 512:(n + 1) * 512], pa[:], wsc[:], xch,
                        op0=ALU.mult, op1=ALU.add,
                    )

                for i in range(4):
                    c0, c1 = i * 1000, (i + 1) * 1000
                    nc.sync.dma_start(out_h[rows, c0:c1], ob[:, c0:c1])

    nc.finalize()
    return nc


def _run(x, w1, b1, w2, b2, gamma, **run_kwargs):
    x = np.ascontiguousarray(np.asarray(x, dtype=np.float32))
    w1 = np.asarray(w1, dtype=np.float32)
    b1 = np.asarray(b1, dtype=np.float32)
    w2 = np.asarray(w2, dtype=np.float32)
    b2 = np.asarray(b2, dtype=np.float32)
    gamma = np.asarray(gamma, dtype=np.float32)
    assert x.shape == (B, 1, C, T), x.shape

    A = float(w1 @ w2)
    B2c = float(b1 @ w2)
    gam = float(gamma.reshape(-1)[0])

    nc = build_program(A, B2c, gam)

    eye = np.eye(128, dtype=np.float32)
    xs = x[:, 0].reshape(N_CORES, SPC * C, T)
    in_maps = [{"x": np.ascontiguousarray(xs[r]), "ident": eye} for r in range(N_CORES)]
    res = run_bass_kernel_spmd(nc, in_maps, list(range(N_CORES)), **run_kwargs)
    out = np.stack([res.results[r]["out"] for r in range(N_CORES)])
    out = out.reshape(B, C, T)[:, None].astype(np.float32)
    return out, res


def kernel(x, w1, b1, w2, b2, gamma):
    out, _ = _run(x, w1, b1, w2, b2, gamma)
    return out


def make_timed_runner(nc, in_maps):
    """Build a jitted 8-core runner (no donation) for repeat timing.

    Mirrors bass2jax.run_bass_via_pjrt's multi-core path but keeps the jitted
    function so the NEFF can be executed repeatedly with device-resident args.
    """
    import jax
    import numpy as _np
    from jax.sharding import Mesh, PartitionSpec
    from jax.experimental.shard_map import shard_map

    import concourse.mybir as _mybir
    from concourse import bass2jax
    from concourse.bass2jax import _bass_exec_p, install_neuronx_cc_hook

    install_neuronx_cc_hook()
    n_cores = len(in_maps)
    partition_name = nc.partition_id_tensor.name if nc.partition_id_tensor else None

    in_names, out_names, out_avals, zero_outs = [], [], [], []
    for alloc in nc.m.functions[0].allocations:
        if not isinstance(alloc, _mybir.MemoryLocationSet):
            continue
        name = alloc.memorylocations[0].name
        if alloc.kind == "ExternalInput":
            if name != partition_name:
                in_names.append(name)
        elif alloc.kind == "ExternalOutput":
            out_names.append(name)
            shape = tuple(alloc.tensor_shape)
            dtype = _mybir.dt.np(alloc.dtype)
            out_avals.append(jax.core.ShapedArray(shape, dtype))
            zero_outs.append(_np.zeros(shape, dtype))
    n_params = len(in_names)
    in_names = in_names + out_names
    if partition_name is not None:
        in_names.append(partition_name)

    def _body(*args):
        operands = list(args)
        if partition_name is not None:
            operands.append(bass2jax.partition_id_tensor())
        outs = _bass_exec_p.bind(
            *operands,
            out_avals=tuple(out_avals),
            in_names=tuple(in_names),
            out_names=tuple(out_names),
            lowering_input_output_aliases=(),
            sim_require_finite=True,
            sim_require_nnan=True,
            nc=nc,
        )
        return tuple(outs)

    devices = jax.devices()[:n_cores]
    mesh = Mesh(_np.asarray(devices), ("core",))
    in_specs = (PartitionSpec("core"),) * (n_params + len(out_names))
    out_specs = (PartitionSpec("core"),) * len(out_names)
    fn = jax.jit(
        shard_map(_body, mesh=mesh, in_specs=in_specs, out_specs=out_specs,
                  check_rep=False),
        keep_unused=True,
    )
    concat_in = [
        _np.concatenate([_np.asarray(in_maps[c][nm]) for c in range(n_cores)], axis=0)
        for nm in in_names[:n_params]
    ]
    concat_zeros = [
        _np.zeros((n_cores * z.shape[0], *z.shape[1:]), z.dtype) for z in zero_outs
    ]
    args = [jax.device_put(a) for a in concat_in + concat_zeros]

    def run():
        return jax.block_until_ready(fn(*args))

    return run, out_names, out_avals


def timed_run(x, w1, b1, w2, b2, gamma, iters=30):
    x = np.ascontiguousarray(np.asarray(x, dtype=np.float32))
    A = float(np.asarray(w1, np.float32) @ np.asarray(w2, np.float32))
    B2c = float(np.asarray(b1, np.float32) @ np.asarray(w2, np.float32))
    gam = float(np.asarray(gamma, np.float32).reshape(-1)[0])
    nc = build_program(A, B2c, gam)
    eye = np.eye(128, dtype=np.float32)
    xs = x[:, 0].reshape(N_CORES, SPC * C, T)
    in_maps = [{"x": np.ascontiguousarray(xs[r]), "ident": eye} for r in range(N_CORES)]
    run, out_names, out_avals = make_timed_runner(nc, in_maps)

    import time as _time
    out_arrs = run()  # compile + warmup
    times = []
    for _ in range(iters):
        t0 = _time.perf_counter_ns()
        run()
        times.append(_time.perf_counter_ns() - t0)
    out = np.asarray(out_arrs[out_names.index("out")])
    out = out.reshape(N_CORES, *out_avals[0].shape).reshape(B, C, T)[:, None]
    return out.astype(np.float32), times


# revision 14
# speedup vs baseline: 184.3767x; 184.3767x over previous
"""ChannelSelfAttn Trainium2 kernel.

Reference computation (per sample b, x_b: [C=64, T=4000]):
    q = w1*x + b1, k = w2*x + b2 broadcast over F=16 feature maps
    e[i,j] = sum_{f,t} q[f,i,t]*k[f,j,t]
           = A*G[i,j] + B1*s_i + B2*s_j + C0*T
      where G = x_b @ x_b.T, s = rowsum(x_b),
            A = w1.w2, B1 = w1.b2, B2 = b1.w2, C0 = b1.b2
    e <- (e - min_j e)/(max_j e - min_j e + 1e-8)   # row terms B1*s_i, C0*T cancel
    e <- softmax_j(e)
    out = gamma * (e @ x_b) + x_b

So only f = A*G + B2*s_j survives the normalize; everything reduces to a
64x64 gram matrix + row-softmax + a second small matmul per sample.

Sharding: data-parallel over batch. 32 samples / 8 cores = 4 samples/core,
processed as 2 pairs; each pair stacks 2 samples' channels into the 128
SBUF partitions.
"""

import numpy as np

import concourse.bacc as bacc
import concourse.bass as bass
import concourse.mybir as mybir
import concourse.tile as tile
from concourse.bass_utils import run_bass_kernel_spmd

FP32 = mybir.dt.float32
AF = mybir.ActivationFunctionType
ALU = mybir.AluOpType
AX = mybir.AxisListType

B, C, T = 32, 64, 4000
N_CORES = 8
SPC = B // N_CORES          # samples per core = 4
PAIRS = SPC // 2            # 2
TPAD = 4096                 # T padded to 32 chunks of 128
NCHUNK = TPAD // 128        # 32
NATT = TPAD // 512          # 8 attention N-chunks


def build_program(A: float, B2: float, gamma: float) -> bass.Bass:
    nc = bacc.Bacc(None)
    x_h = nc.declare_dram_parameter("x", [SPC * C, T], FP32, isOutput=False)
    id_h = nc.declare_dram_parameter("ident", [128, 128], FP32, isOutput=False)
    out_h = nc.declare_dram_parameter("out", [SPC * C, T], FP32, isOutput=True)

    with tile.TileContext(nc) as tc:
        with (
            tc.tile_pool(name="xio", bufs=2) as p_x,
            tc.tile_pool(name="xT", bufs=2) as p_xT,
            tc.tile_pool(name="outb", bufs=2) as p_out,
            tc.tile_pool(name="small", bufs=2) as p_small,
            tc.tile_pool(name="const", bufs=1) as p_const,
            tc.tile_pool(name="pt", bufs=2, space="PSUM") as p_pt,
            tc.tile_pool(name="pg", bufs=2, space="PSUM") as p_pg,
            tc.tile_pool(name="pa", bufs=2, space="PSUM") as p_pa,
            tc.tile_pool(name="ps", bufs=2, space="PSUM") as p_ps,
        ):
            ident = p_const.tile([128, 128], FP32)
            nc.sync.dma_start(ident[:], id_h[:, :])
            ones = p_const.tile([1, 128], FP32)
            nc.vector.memset(ones[:], 1.0)

            for p in range(PAIRS):
                rows = slice(p * 128, (p + 1) * 128)

                # ---- load x pair [128, 4000], zero-pad t to 4096
                x_stack = p_x.tile([128, TPAD], FP32)
                for i in range(4):
                    c0, c1 = i * 1000, (i + 1) * 1000
                    nc.sync.dma_start(x_stack[:, c0:c1], x_h[rows, c0:c1])
                nc.vector.memset(x_stack[:, T:TPAD], 0.0)

                # ---- transpose to xT [t, c] chunks via PE
                xT = p_xT.tile([128, TPAD], FP32)
                for q in range(NCHUNK // 4):
                    pt = p_pt.tile([128, 512], FP32, tag="pt")
                    for j in range(4):
                        k = 4 * q + j
                        nc.tensor.transpose(
                            pt[:, j * 128:(j + 1) * 128],
                            x_stack[:, k * 128:(k + 1) * 128],
                            ident[:, :],
                        )
                    dst = xT[:, q * 512:(q + 1) * 512]
                    if q % 2 == 0:
                        nc.vector.tensor_copy(dst, pt[:])
                    else:
                        nc.scalar.copy(dst, pt[:])

                # ---- gram: pg = sum_k xT_k.T @ xT_k  -> [[G_A, .], [., G_B]]
                pg = p_pg.tile([128, 128], FP32, tag="pg")
                for k in range(NCHUNK):
                    blkk = xT[:, k * 128:(k + 1) * 128]
                    nc.tensor.matmul(
                        pg[:], lhsT=blkk, rhs=blkk,
                        start=(k == 0), stop=(k == NCHUNK - 1),
                    )

                # ---- s = rowsum(x), then broadcast B2*s as a row
                s_col = p_small.tile([128, 1], FP32, tag="scol")
                nc.vector.reduce_sum(s_col[:], x_stack[:], axis=AX.X)
                ps_row = p_ps.tile([1, 128], FP32, tag="ps")
                nc.tensor.transpose(ps_row[:], s_col[:], ident[:, :])
                srow_b = p_small.tile([1, 128], FP32, tag="srowb")
                nc.scalar.mul(srow_b[:], ps_row[:], B2)
                # broadcast row down all partitions: psb[m, j] = B2*s_j
                psb = p_ps.tile([128, 128], FP32, tag="ps")
                nc.tensor.matmul(psb[:], lhsT=ones[:], rhs=srow_b[:], start=True, stop=True)
                sbs = p_small.tile([128, C], FP32, tag="sbs")
                nc.scalar.copy(sbs[0:64, :], psb[0:64, 0:64])
                nc.scalar.copy(sbs[64:128, :], psb[64:128, 64:128])

                # ---- f = A*G + B2*s_j (diag blocks only), stacked [128, 64]
                fs = p_small.tile([128, C], FP32, tag="fs")
                nc.vector.scalar_tensor_tensor(
                    fs[0:64, :], pg[0:64, 0:64], A, sbs[0:64, :],
                    op0=ALU.mult, op1=ALU.add,
                )
                nc.vector.scalar_tensor_tensor(
                    fs[64:128, :], pg[64:128, 64:128], A, sbs[64:128, :],
                    op0=ALU.mult, op1=ALU.add,
                )

                # ---- row minmax-normalize + exp (+ rowsum for softmax denom)
                mx = p_small.tile([128, 1], FP32, tag="mx")
                nc.vector.reduce_max(mx[:], fs[:], axis=AX.X)
                mn = p_small.tile([128, 1], FP32, tag="mn")
                nc.vector.tensor_reduce(mn[:], fs[:], axis=AX.X, op=ALU.min)
                dd = p_small.tile([128, 1], FP32, tag="dd")
                nc.vector.scalar_tensor_tensor(
                    dd[:], mx[:], 1e-8, mn[:], op0=ALU.add, op1=ALU.subtract,
                )
                rr = p_small.tile([128, 1], FP32, tag="rr")
                nc.vector.reciprocal(rr[:], dd[:])
                nb = p_small.tile([128, 1], FP32, tag="nb")
                nc.vector.scalar_tensor_tensor(
                    nb[:], mn[:], -1.0, rr[:], op0=ALU.mult, op1=ALU.mult,
                )
                # exp() written into diagonal blocks of a zeroed [128,128] tile
                # so ONE transpose yields the block-diag lhsT for the attn matmul
                gw = p_small.tile([128, 128], FP32, tag="gw")
                nc.vector.memset(gw[:], 0.0)
                se = p_small.tile([128, 1], FP32, tag="se")
                nc.scalar.activation(
                    gw[0:64, 0:64], fs[0:64, :], AF.Exp,
                    bias=nb[0:64], scale=rr[0:64], accum_out=se[0:64],
                )
                nc.scalar.activation(
                    gw[64:128, 64:128], fs[64:128, :], AF.Exp,
                    bias=nb[64:128], scale=rr[64:128], accum_out=se[64:128],
                )
                rs = p_small.tile([128, 1], FP32, tag="rs")
                nc.vector.reciprocal(rs[:], se[:])
                wsc = p_small.tile([128, 1], FP32, tag="wsc")
                nc.vector.tensor_scalar_mul(wsc[:], rs[:], gamma)

                # ---- block-diag transposed weights for attn matmul
                pb = p_ps.tile([128, 128], FP32, tag="ps")
                nc.tensor.transpose(pb[:], gw[:], ident[:, :])
                blk = p_small.tile([128, 128], FP32, tag="blk")
                nc.vector.tensor_copy(blk[:], pb[:])

                # ---- attn = blk.T @ x (unnormalized), out = wsc*attn + x
                ob = p_out.tile([128, TPAD], FP32)
                for n in range(NATT):
                    pa = p_pa.tile([128, 512], FP32, tag="pa")
                    xch = x_stack[:, n * 512:(n + 1) * 512]
                    nc.tensor.matmul(pa[:], lhsT=blk[:], rhs=xch, start=True, stop=True)
                    nc.vector.scalar_tensor_tensor(
                        ob[:, n * 512:(n + 1) * 512], pa[:], wsc[:], xch,
                        op0=ALU.mult, op1=ALU.add,
                    )

                for i in range(4):
                    c0, c1 = i * 1000, (i + 1) * 1000
                    nc.sync.dma_start(out_h[rows, c0:c1], ob[:, c0:c1])

    nc.finalize()
    return nc


def _run(x, w1, b1, w2, b2, gamma, **run_kwargs):
    x = np.ascontiguousarray(np.asarray(x, dtype=np.float32))
    w1 = np.asarray(w1, dtype=np.float32)
    b1 = np.asarray(b1, dtype=np.float32)
    w2 = np.asarray(w2, dtype=np.float32)
    b2 = np.asarray(b2, dtype=np.float32)
    gamma = np.asarray(gamma, dtype=np.float32)
    assert x.shape == (B, 1, C, T), x.shape

    A = float(w1 @ w2)
    B2c = float(b1 @ w2)
    gam = float(gamma.reshape(-1)[0])

    nc = build_program(A, B2c, gam)

    eye = np.eye(128, dtype=np.float32)
    xs = x[:, 0].reshape(N_CORES, SPC * C, T)
    in_maps = [{"x": np.ascontiguousarray(xs[r]), "ident": eye} for r in range(N_CORES)]
    res = run_bass_kernel_spmd(nc, in_maps, list(range(N_CORES)), **run_kwargs)
    out = np.stack([res.results[r]["out"] for r in range(N_CORES)])
    out = out.reshape(B, C, T)[:, None].astype(np.float32)
    return out, res


def kernel(x, w1, b1, w2, b2, gamma):
    out, _ = _run(x, w1, b1, w2, b2, gamma)
    return out


def make_timed_runner(nc, in_maps):
    """Build a jitted 8-core runner (no donation) for repeat timing.

    Mirrors bass2jax.run_bass_via_pjrt's multi-core path but keeps the jitted
    function so the NEFF can be executed repeatedly with device-resident args.
    """
    import jax
    import numpy as _np
    from jax.sharding import Mesh, PartitionSpec
    from jax.experimental.shard_map import shard_map

    import concourse.mybir as _mybir
    from concourse import bass2jax
    from concourse.bass2jax import _bass_exec_p, install_neuronx_cc_hook

    install_neuronx_cc_hook()
    n_cores = len(in_maps)
    partition_name = nc.partition_id_tensor.name if nc.partition_id_tensor else None

    in_names, out_names, out_avals, zero_outs = [], [], [], []
    for alloc in nc.m.functions[0].allocations:
        if not isinstance(alloc, _mybir.MemoryLocationSet):
            continue
        name = alloc.memorylocations[0].name
        if alloc.kind == "ExternalInput":
            if name != partition_name:
                in_names.append(name)
        elif alloc.kind == "ExternalOutput":
            out_names.append(name)
            shape = tuple(alloc.tensor_shape)
            dtype = _mybir.dt.np(alloc.dtype)
            out_avals.append(jax.core.ShapedArray(shape, dtype))
            zero_outs.append(_np.zeros(shape, dtype))
    n_params = len(in_names)
    in_names = in_names + out_names
    if partition_name is not None:
        in_names.append(partition_name)

    def _body(*args):
        operands = list(args)
        if partition_name is not None:
            operands.append(bass2jax.partition_id_tensor())
        outs = _bass_exec_p.bind(
            *operands,
            out_avals=tuple(out_avals),
            in_names=tuple(in_names),
            out_names=tuple(out_names),
            lowering_input_output_aliases=(),
            sim_require_finite=True,
            sim_require_nnan=True,
            nc=nc,
        )
        return tuple(outs)

    devices = jax.devices()[:n_cores]
    mesh = Mesh(_np.asarray(devices), ("core",))
    in_specs = (PartitionSpec("core"),) * (n_params + len(out_names))
    out_specs = (PartitionSpec("core"),) * len(out_names)
    fn = jax.jit(
        shard_map(_body, mesh=mesh, in_specs=in_specs, out_specs=out_specs,
                  check_rep=False),
        keep_unused=True,
    )
    concat_in = [
        _np.concatenate([_np.asarray(in_maps[c][nm]) for c in range(n_cores)], axis=0)
        for nm in in_names[:n_params]
    ]
    concat_zeros = [
        _np.zeros((n_cores * z.shape[0], *z.shape[1:]), z.dtype) for z in zero_outs
    ]
    shard = jax.sharding.NamedSharding(mesh, PartitionSpec("core"))
    args = [jax.device_put(a, shard) for a in concat_in + concat_zeros]

    assert len(out_names) == 1

    def run_chain(k):
        """k chained executions (each consumes the previous output as its
        donated-out operand) dispatched async; blocks at the end."""
        o = args[-1]
        for _ in range(k):
            o = fn(*args[:n_params], o)[0]
        return jax.block_until_ready(o)

    return run_chain, out_names, out_avals


def timed_run(x, w1, b1, w2, b2, gamma, iters=30):
    x = np.ascontiguousarray(np.asarray(x, dtype=np.float32))
    A = float(np.asarray(w1, np.float32) @ np.asarray(w2, np.float32))
    B2c = float(np.asarray(b1, np.float32) @ np.asarray(w2, np.float32))
    gam = float(np.asarray(gamma, np.float32).reshape(-1)[0])
    nc = build_program(A, B2c, gam)
    eye = np.eye(128, dtype=np.float32)
    xs = x[:, 0].reshape(N_CORES, SPC * C, T)
    in_maps = [{"x": np.ascontiguousarray(xs[r]), "ident": eye} for r in range(N_CORES)]
    run_chain, out_names, out_avals = make_timed_runner(nc, in_maps)

    import time as _time
    out_arr = run_chain(1)  # compile + warmup
    run_chain(1)

    def t_of(k, reps=5):
        best = None
        for _ in range(reps):
            t0 = _time.perf_counter_ns()
            run_chain(k)
            dt = _time.perf_counter_ns() - t0
            best = dt if best is None else min(best, dt)
        return best

    k1, k2 = 2, 2 + iters
    t1 = t_of(k1)
    t2 = t_of(k2)
    per_exec_ns = (t2 - t1) / (k2 - k1)
    out = np.asarray(out_arr)
    out = out.reshape(N_CORES, *out_avals[0].shape).reshape(B, C, T)[:, None]
    return out.astype(np.float32), per_exec_ns


# revision 41
# speedup vs baseline: 2097.1538x; 11.3743x over previous
"""ChannelSelfAttn Trainium2 kernel.

Reference computation (per sample b, x_b: [C=64, T=4000]):
    q = w1*x + b1, k = w2*x + b2 broadcast over F=16 feature maps
    e[i,j] = sum_{f,t} q[f,i,t]*k[f,j,t]
           = A*G[i,j] + B1*s_i + B2*s_j + C0*T
      where G = x_b @ x_b.T, s = rowsum(x_b),
            A = w1.w2, B1 = w1.b2, B2 = b1.w2, C0 = b1.b2
    e <- (e - min_j e)/(max_j e - min_j e + 1e-8)   # row terms B1*s_i, C0*T cancel
    e <- softmax_j(e)
    out = gamma * (e @ x_b) + x_b

So only f = A*G + B2*s_j survives the normalize; everything reduces to a
64x64 gram matrix + row-softmax + a second small matmul per sample.

Sharding: data-parallel over batch. 32 samples / 8 cores = 4 samples/core,
processed as 2 pairs; each pair stacks 2 samples' channels into the 128
SBUF partitions.
"""

import numpy as np

import concourse.bacc as bacc
import concourse.bass as bass
import concourse.mybir as mybir
import concourse.tile as tile
from concourse.bass_utils import run_bass_kernel_spmd

FP32 = mybir.dt.float32
AF = mybir.ActivationFunctionType
ALU = mybir.AluOpType
AX = mybir.AxisListType

B, C, T = 32, 64, 4000
N_CORES = 8
SPC = B // N_CORES          # samples per core = 4
PAIRS = SPC // 2            # 2
TPAD = 4096                 # T padded to 32 chunks of 128
NCHUNK = TPAD // 128        # 32
NATT = TPAD // 512          # 8 attention N-chunks


def build_program(A: float, B2: float, gamma: float, replicate: int = 1,
                  f32r_attn: bool = False, f32r_trans: bool = False,
                  f32r_gram: bool = False) -> bass.Bass:
    FP32R = mybir.dt.float32r

    def ra(ap):  # attn operands
        return ap.bitcast(FP32R) if f32r_attn else ap

    def rt(ap):  # transpose operands
        return ap.bitcast(FP32R) if f32r_trans else ap

    def rg(ap):  # gram operands
        return ap.bitcast(FP32R) if f32r_gram else ap

    nc = bacc.Bacc(None)
    x_h = nc.declare_dram_parameter("x", [SPC * C, T], FP32, isOutput=False)
    id_h = nc.declare_dram_parameter("ident", [128, 128], FP32, isOutput=False)
    out_h = nc.declare_dram_parameter("out", [SPC * C, T], FP32, isOutput=True)

    with tile.TileContext(nc) as tc:
        with (
            tc.tile_pool(name="xio", bufs=2) as p_x,
            tc.tile_pool(name="xT", bufs=2) as p_xT,
            tc.tile_pool(name="outb", bufs=2) as p_out,
            tc.tile_pool(name="small", bufs=2) as p_small,
            tc.tile_pool(name="const", bufs=1) as p_const,
            tc.tile_pool(name="pt", bufs=2, space="PSUM") as p_pt,
            tc.tile_pool(name="pg", bufs=2, space="PSUM") as p_pg,
            tc.tile_pool(name="pa", bufs=2, space="PSUM") as p_pa,
            tc.tile_pool(name="ps", bufs=2, space="PSUM") as p_ps,
        ):
            ident = p_const.tile([128, 128], FP32)
            nc.sync.dma_start(ident[:], id_h[:, :])
            ones = p_const.tile([1, 128], FP32)
            nc.vector.memset(ones[:], 1.0)

            for p in [pp for _ in range(replicate) for pp in range(PAIRS)]:
                rows = slice(p * 128, (p + 1) * 128)

                # ---- load x pair [128, 4000], zero-pad t to 4096
                x_stack = p_x.tile([128, TPAD], FP32)
                for i in range(4):
                    c0, c1 = i * 1000, (i + 1) * 1000
                    nc.sync.dma_start(x_stack[:, c0:c1], x_h[rows, c0:c1])
                nc.vector.memset(x_stack[:, T:TPAD], 0.0)

                # ---- transpose to xT [t, c] chunks via PE
                xT = p_xT.tile([128, TPAD], FP32)
                for q in range(NCHUNK // 4):
                    pt = p_pt.tile([128, 512], FP32, tag="pt")
                    for j in range(4):
                        k = 4 * q + j
                        nc.tensor.transpose(
                            rt(pt[:, j * 128:(j + 1) * 128]),
                            rt(x_stack[:, k * 128:(k + 1) * 128]),
                            rt(ident[:, :]),
                        )
                    dst = xT[:, q * 512:(q + 1) * 512]
                    if q % 2 == 0:
                        nc.vector.tensor_copy(dst, pt[:])
                    else:
                        nc.scalar.copy(dst, pt[:])

                # ---- gram: pg = sum_k xT_k.T @ xT_k  -> [[G_A, .], [., G_B]]
                pg = p_pg.tile([128, 128], FP32, tag="pg")
                for k in range(NCHUNK):
                    blkk = rg(xT[:, k * 128:(k + 1) * 128])
                    nc.tensor.matmul(
                        pg[:], lhsT=blkk, rhs=blkk,
                        start=(k == 0), stop=(k == NCHUNK - 1),
                    )

                # ---- s = rowsum(x), then broadcast B2*s as a row
                s_col = p_small.tile([128, 1], FP32, tag="scol")
                nc.vector.reduce_sum(s_col[:], x_stack[:], axis=AX.X)
                ps_row = p_ps.tile([1, 128], FP32, tag="ps")
                nc.tensor.transpose(ps_row[:], s_col[:], ident[:, :])
                srow_b = p_small.tile([1, 128], FP32, tag="srowb")
                nc.scalar.mul(srow_b[:], ps_row[:], B2)
                # broadcast row down all partitions: psb[m, j] = B2*s_j
                psb = p_ps.tile([128, 128], FP32, tag="ps")
                nc.tensor.matmul(psb[:], lhsT=ones[:], rhs=srow_b[:], start=True, stop=True)
                sbs = p_small.tile([128, C], FP32, tag="sbs")
                nc.scalar.copy(sbs[0:64, :], psb[0:64, 0:64])
                nc.scalar.copy(sbs[64:128, :], psb[64:128, 64:128])

                # ---- f = A*G + B2*s_j (diag blocks only), stacked [128, 64]
                fs = p_small.tile([128, C], FP32, tag="fs")
                nc.vector.scalar_tensor_tensor(
                    fs[0:64, :], pg[0:64, 0:64], A, sbs[0:64, :],
                    op0=ALU.mult, op1=ALU.add,
                )
                nc.vector.scalar_tensor_tensor(
                    fs[64:128, :], pg[64:128, 64:128], A, sbs[64:128, :],
                    op0=ALU.mult, op1=ALU.add,
                )

                # ---- row minmax-normalize + exp (+ rowsum for softmax denom)
                mx = p_small.tile([128, 1], FP32, tag="mx")
                nc.vector.reduce_max(mx[:], fs[:], axis=AX.X)
                mn = p_small.tile([128, 1], FP32, tag="mn")
                nc.vector.tensor_reduce(mn[:], fs[:], axis=AX.X, op=ALU.min)
                dd = p_small.tile([128, 1], FP32, tag="dd")
                nc.vector.scalar_tensor_tensor(
                    dd[:], mx[:], 1e-8, mn[:], op0=ALU.add, op1=ALU.subtract,
                )
                rr = p_small.tile([128, 1], FP32, tag="rr")
                nc.vector.reciprocal(rr[:], dd[:])
                nb = p_small.tile([128, 1], FP32, tag="nb")
                nc.vector.scalar_tensor_tensor(
                    nb[:], mn[:], -1.0, rr[:], op0=ALU.mult, op1=ALU.mult,
                )
                # exp() written into diagonal blocks of a zeroed [128,128] tile
                # so ONE transpose yields the block-diag lhsT for the attn matmul
                gw = p_small.tile([128, 128], FP32, tag="gw")
                nc.vector.memset(gw[:], 0.0)
                se = p_small.tile([128, 1], FP32, tag="se")
                nc.scalar.activation(
                    gw[0:64, 0:64], fs[0:64, :], AF.Exp,
                    bias=nb[0:64], scale=rr[0:64], accum_out=se[0:64],
                )
                nc.scalar.activation(
                    gw[64:128, 64:128], fs[64:128, :], AF.Exp,
                    bias=nb[64:128], scale=rr[64:128], accum_out=se[64:128],
                )
                rs = p_small.tile([128, 1], FP32, tag="rs")
                nc.vector.reciprocal(rs[:], se[:])
                wsc = p_small.tile([128, 1], FP32, tag="wsc")
                nc.vector.tensor_scalar_mul(wsc[:], rs[:], gamma)

                # ---- block-diag transposed weights for attn matmul
                pb = p_ps.tile([128, 128], FP32, tag="ps")
                nc.tensor.transpose(pb[:], gw[:], ident[:, :])
                blk = p_small.tile([128, 128], FP32, tag="blk")
                nc.vector.tensor_copy(blk[:], pb[:])

                # ---- attn = blk.T @ x (unnormalized), out = wsc*attn + x
                ob = p_out.tile([128, TPAD], FP32)
                for n in range(NATT):
                    pa = p_pa.tile([128, 512], FP32, tag="pa")
                    xch = x_stack[:, n * 512:(n + 1) * 512]
                    nc.tensor.matmul(pa[:], lhsT=ra(blk[:]), rhs=ra(xch),
                                     start=True, stop=True)
                    nc.vector.scalar_tensor_tensor(
                        ob[:, n * 512:(n + 1) * 512], pa[:], wsc[:], xch,
                        op0=ALU.mult, op1=ALU.add,
                    )

                for i in range(4):
                    c0, c1 = i * 1000, (i + 1) * 1000
                    nc.sync.dma_start(out_h[rows, c0:c1], ob[:, c0:c1])

    nc.finalize()
    return nc


def build_program_f32r(A: float, B2: float, gamma: float, replicate: int = 1,
                       fuse_s: bool = True, dma_pieces: int = 4,
                       xr_gp: bool = False, pa3: bool = False,
                       exact_g: bool = False, ring_split: bool = False) -> bass.Bass:
    """float32r variant: matmul/transpose operands in fp32r (reduced-precision
    fp32 that streams at full PE rate for moving>=256). All f32r operands are
    produced by compute ops (rounding); the +x residual stays exact fp32.
    Gram uses a 256-wide moving window (right half is discarded garbage) to
    hit the f32r full-rate threshold. With fuse_s, xT chunks are laid out at
    stride 129 with a ones column appended, so PSUM column 128 of the gram
    accumulates the row-sum s for free (no big DVE reduction)."""
    FP32R = mybir.dt.float32r
    GDT = FP32 if exact_g else FP32R     # dtype of transpose/gram pipeline
    GW_RHS = (129 if exact_g else 256) if fuse_s else (128 if exact_g else 256)
    CW = 129 if fuse_s else 128          # xT column stride per chunk
    XTW = NCHUNK * CW + max(GW_RHS - CW, 0)

    nc = bacc.Bacc(None)
    x_h = nc.declare_dram_parameter("x", [SPC * C, T], FP32, isOutput=False)
    id_h = nc.declare_dram_parameter("ident", [128, 128], FP32, isOutput=False)
    out_h = nc.declare_dram_parameter("out", [SPC * C, T], FP32, isOutput=True)

    with tile.TileContext(nc) as tc:
        with (
            tc.tile_pool(name="xio", bufs=2) as p_x,
            tc.tile_pool(name="xr", bufs=2) as p_xr,
            tc.tile_pool(name="xT", bufs=2) as p_xT,
            tc.tile_pool(name="outb", bufs=2) as p_out,
            tc.tile_pool(name="small", bufs=2) as p_small,
            tc.tile_pool(name="const", bufs=1) as p_const,
            tc.tile_pool(name="pt", bufs=2, space="PSUM") as p_pt,
            tc.tile_pool(name="pg", bufs=2, space="PSUM") as p_pg,
            tc.tile_pool(name="pa", bufs=3 if pa3 else 2, space="PSUM") as p_pa,
            tc.tile_pool(name="ps", bufs=1 if pa3 else 2, space="PSUM") as p_ps,
        ):
            ident = p_const.tile([128, 128], FP32)
            nc.sync.dma_start(ident[:], id_h[:, :])
            identr = p_const.tile([128, 128], FP32R)
            nc.vector.tensor_copy(identr[:], ident[:])
            ones_f = p_const.tile([1, 128], FP32)
            nc.vector.memset(ones_f[:], 1.0)
            ones = p_const.tile([1, 128], FP32R)
            nc.vector.tensor_copy(ones[:], ones_f[:])
            zsrc = p_const.tile([128, 128], FP32)
            nc.vector.memset(zsrc[:], 0.0)
            c_ones = p_const.tile([128, NCHUNK], FP32)
            nc.vector.memset(c_ones[:], 1.0)

            for p in [pp for _ in range(replicate) for pp in range(PAIRS)]:
                rows = slice(p * 128, (p + 1) * 128)

                # ---- load x pair [128, 4000] fp32, zero-pad t to 4096
                x_stack = p_x.tile([128, TPAD], FP32)
                npc = T // dma_pieces
                for i in range(dma_pieces):
                    c0, c1 = i * npc, (i + 1) * npc
                    eng = nc.scalar if (ring_split and i % 2) else nc.sync
                    eng.dma_start(x_stack[:, c0:c1], x_h[rows, c0:c1])
                nc.vector.memset(x_stack[:, T:TPAD], 0.0)

                # ---- rounded fp32r copy (feeds every matmul operand)
                xr = p_xr.tile([128, TPAD], FP32R)
                if xr_gp:
                    nc.gpsimd.tensor_copy(xr[:, 0:2048], x_stack[:, 0:2048])
                    nc.gpsimd.tensor_copy(xr[:, 2048:TPAD], x_stack[:, 2048:TPAD])
                else:
                    nc.vector.tensor_copy(xr[:, 0:2048], x_stack[:, 0:2048])
                    nc.scalar.copy(xr[:, 2048:TPAD], x_stack[:, 2048:TPAD])

                # ---- transpose to xT [t, c] chunks via PE
                xT = p_xT.tile([128, XTW], GDT)
                if XTW > NCHUNK * CW:
                    nc.vector.tensor_copy(xT[:, NCHUNK * CW:XTW], zsrc[:, 0:XTW - NCHUNK * CW])
                if fuse_s:
                    # ones column after each chunk's 128 data columns
                    oview = xT[:, 0:NCHUNK * CW].rearrange("p (k c) -> p k c", c=CW)
                    nc.vector.tensor_copy(oview[:, :, 128:129], c_ones[:].rearrange("p (k o) -> p k o", o=1))
                tsrc = x_stack if exact_g else xr
                tid = ident if exact_g else identr
                for q in range(NCHUNK // 4):
                    pt = p_pt.tile([128, 512], GDT, tag="pt")
                    for j in range(4):
                        k = 4 * q + j
                        nc.tensor.transpose(
                            pt[:, j * 128:(j + 1) * 128],
                            tsrc[:, k * 128:(k + 1) * 128],
                            tid[:, :],
                        )
                    if fuse_s:
                        dst = xT[:, q * 4 * CW:(q * 4 + 4) * CW].rearrange(
                            "p (k c) -> p k c", c=CW)[:, :, 0:128]
                        src = pt[:].rearrange("p (k c) -> p k c", c=128)
                    else:
                        dst = xT[:, q * 512:(q + 1) * 512]
                        src = pt[:]
                    if q % 2 == 0:
                        nc.vector.tensor_copy(dst, src)
                    else:
                        nc.scalar.copy(dst, src)

                # ---- gram, 256-wide moving (full f32r rate); left half valid.
                # With fuse_s, psum col 128 accumulates s = rowsum(x) as well.
                pg = p_pg.tile([128, GW_RHS], FP32, tag="pg")
                for k in range(NCHUNK):
                    nc.tensor.matmul(
                        pg[:], lhsT=xT[:, k * CW:k * CW + 128],
                        rhs=xT[:, k * CW:k * CW + GW_RHS],
                        start=(k == 0), stop=(k == NCHUNK - 1),
                    )

                # ---- B2*s as a row via transpose + rank-1 broadcast matmul
                s_colr = p_small.tile([128, 1], GDT, tag="scolr")
                if fuse_s:
                    nc.scalar.mul(s_colr[:], pg[:, 128:129], B2)
                else:
                    s_col = p_small.tile([128, 1], FP32, tag="scol")
                    nc.vector.reduce_sum(s_col[:], x_stack[:], axis=AX.X)
                    nc.scalar.mul(s_colr[:], s_col[:], B2)
                ps_row = p_ps.tile([1, 128], GDT, tag="ps")
                nc.tensor.transpose(ps_row[:], s_colr[:], tid[:, :])
                srow_b = p_small.tile([1, 128], GDT, tag="srowb")
                nc.vector.tensor_copy(srow_b[:], ps_row[:])
                psb = p_ps.tile([128, 128], FP32, tag="ps")
                nc.tensor.matmul(psb[:], lhsT=(ones_f if exact_g else ones)[:],
                                 rhs=srow_b[:], start=True, stop=True)
                sbs = p_small.tile([128, C], FP32, tag="sbs")
                nc.scalar.copy(sbs[0:64, :], psb[0:64, 0:64])
                nc.scalar.copy(sbs[64:128, :], psb[64:128, 64:128])

                # ---- f = A*G + B2*s_j (diag blocks only), stacked [128, 64]
                fs = p_small.tile([128, C], FP32, tag="fs")
                nc.vector.scalar_tensor_tensor(
                    fs[0:64, :], pg[0:64, 0:64], A, sbs[0:64, :],
                    op0=ALU.mult, op1=ALU.add,
                )
                nc.vector.scalar_tensor_tensor(
                    fs[64:128, :], pg[64:128, 64:128], A, sbs[64:128, :],
                    op0=ALU.mult, op1=ALU.add,
                )

                # ---- row minmax-normalize + exp (+ rowsum for softmax denom)
                mx = p_small.tile([128, 1], FP32, tag="mx")
                nc.vector.reduce_max(mx[:], fs[:], axis=AX.X)
                mn = p_small.tile([128, 1], FP32, tag="mn")
                nc.vector.tensor_reduce(mn[:], fs[:], axis=AX.X, op=ALU.min)
                dd = p_small.tile([128, 1], FP32, tag="dd")
                nc.vector.scalar_tensor_tensor(
                    dd[:], mx[:], 1e-8, mn[:], op0=ALU.add, op1=ALU.subtract,
                )
                rr = p_small.tile([128, 1], FP32, tag="rr")
                nc.vector.reciprocal(rr[:], dd[:])
                nb = p_small.tile([128, 1], FP32, tag="nb")
                nc.vector.scalar_tensor_tensor(
                    nb[:], mn[:], -1.0, rr[:], op0=ALU.mult, op1=ALU.mult,
                )
                gw = p_small.tile([128, 128], FP32R, tag="gw")
                nc.vector.tensor_copy(gw[:], zsrc[:])
                se = p_small.tile([128, 1], FP32, tag="se")
                nc.scalar.activation(
                    gw[0:64, 0:64], fs[0:64, :], AF.Exp,
                    bias=nb[0:64], scale=rr[0:64], accum_out=se[0:64],
                )
                nc.scalar.activation(
                    gw[64:128, 64:128], fs[64:128, :], AF.Exp,
                    bias=nb[64:128], scale=rr[64:128], accum_out=se[64:128],
                )
                rs = p_small.tile([128, 1], FP32, tag="rs")
                nc.vector.reciprocal(rs[:], se[:])
                wsc = p_small.tile([128, 1], FP32, tag="wsc")
                nc.vector.tensor_scalar_mul(wsc[:], rs[:], gamma)

                # ---- block-diag transposed weights for attn matmul
                pb = p_ps.tile([128, 128], FP32R, tag="ps")
                nc.tensor.transpose(pb[:], gw[:], identr[:, :])
                blk = p_small.tile([128, 128], FP32R, tag="blk")
                nc.vector.tensor_copy(blk[:], pb[:])

                # ---- attn = blk.T @ xr (unnormalized), out = wsc*attn + x
                ob = p_out.tile([128, TPAD], FP32)
                for n in range(NATT):
                    pa = p_pa.tile([128, 512], FP32, tag="pa")
                    nc.tensor.matmul(pa[:], lhsT=blk[:], rhs=xr[:, n * 512:(n + 1) * 512],
                                     start=True, stop=True)
                    nc.vector.scalar_tensor_tensor(
                        ob[:, n * 512:(n + 1) * 512], pa[:], wsc[:],
                        x_stack[:, n * 512:(n + 1) * 512],
                        op0=ALU.mult, op1=ALU.add,
                    )

                for i in range(dma_pieces):
                    c0, c1 = i * npc, (i + 1) * npc
                    eng = nc.scalar if (ring_split and i % 2 == 0) else nc.sync
                    eng.dma_start(out_h[rows, c0:c1], ob[:, c0:c1])

    nc.finalize()
    return nc


# Final kernel configuration (selected by on-hardware benchmarking)
BUILD = build_program_f32r
BUILD_KWARGS = {"fuse_s": True}


def _run(x, w1, b1, w2, b2, gamma, **run_kwargs):
    x = np.ascontiguousarray(np.asarray(x, dtype=np.float32))
    w1 = np.asarray(w1, dtype=np.float32)
    b1 = np.asarray(b1, dtype=np.float32)
    w2 = np.asarray(w2, dtype=np.float32)
    b2 = np.asarray(b2, dtype=np.float32)
    gamma = np.asarray(gamma, dtype=np.float32)
    assert x.shape == (B, 1, C, T), x.shape

    A = float(w1 @ w2)
    B2c = float(b1 @ w2)
    gam = float(gamma.reshape(-1)[0])

    nc = BUILD(A, B2c, gam, **BUILD_KWARGS)

    eye = np.eye(128, dtype=np.float32)
    xs = x[:, 0].reshape(N_CORES, SPC * C, T)
    in_maps = [{"x": np.ascontiguousarray(xs[r]), "ident": eye} for r in range(N_CORES)]
    res = run_bass_kernel_spmd(nc, in_maps, list(range(N_CORES)), **run_kwargs)
    out = np.stack([res.results[r]["out"] for r in range(N_CORES)])
    out = out.reshape(B, C, T)[:, None].astype(np.float32)
    return out, res


def kernel(x, w1, b1, w2, b2, gamma):
    out, _ = _run(x, w1, b1, w2, b2, gamma)
    return out


def make_timed_runner(nc, in_maps):
    """Build a jitted 8-core runner (no donation) for repeat timing.

    Mirrors bass2jax.run_bass_via_pjrt's multi-core path but keeps the jitted
    function so the NEFF can be executed repeatedly with device-resident args.
    """
    import jax
    import numpy as _np
    from jax.sharding import Mesh, PartitionSpec
    from jax.experimental.shard_map import shard_map

    import concourse.mybir as _mybir
    from concourse import bass2jax
    from concourse.bass2jax import _bass_exec_p, install_neuronx_cc_hook

    install_neuronx_cc_hook()
    n_cores = len(in_maps)
    partition_name = nc.partition_id_tensor.name if nc.partition_id_tensor else None

    in_names, out_names, out_avals, zero_outs = [], [], [], []
    for alloc in nc.m.functions[0].allocations:
        if not isinstance(alloc, _mybir.MemoryLocationSet):
            continue
        name = alloc.memorylocations[0].name
        if alloc.kind == "ExternalInput":
            if name != partition_name:
                in_names.append(name)
        elif alloc.kind == "ExternalOutput":
            out_names.append(name)
            shape = tuple(alloc.tensor_shape)
            dtype = _mybir.dt.np(alloc.dtype)
            out_avals.append(jax.core.ShapedArray(shape, dtype))
            zero_outs.append(_np.zeros(shape, dtype))
    n_params = len(in_names)
    in_names = in_names + out_names
    if partition_name is not None:
        in_names.append(partition_name)

    def _exec_once(*args):
        operands = list(args)
        if partition_name is not None:
            operands.append(bass2jax.partition_id_tensor())
        outs = _bass_exec_p.bind(
            *operands,
            out_avals=tuple(out_avals),
            in_names=tuple(in_names),
            out_names=tuple(out_names),
            lowering_input_output_aliases=(),
            sim_require_finite=True,
            sim_require_nnan=True,
            nc=nc,
        )
        return tuple(outs)

    assert len(out_names) == 1

    devices = jax.devices()[:n_cores]
    mesh = Mesh(_np.asarray(devices), ("core",))
    in_specs = (PartitionSpec("core"),) * (n_params + len(out_names))
    out_specs = (PartitionSpec("core"),) * len(out_names)
    fn = jax.jit(
        shard_map(_exec_once, mesh=mesh, in_specs=in_specs, out_specs=out_specs,
                  check_rep=False),
        keep_unused=True,
    )
    concat_in = [
        _np.concatenate([_np.asarray(in_maps[c][nm]) for c in range(n_cores)], axis=0)
        for nm in in_names[:n_params]
    ]
    concat_zeros = [
        _np.zeros((n_cores * z.shape[0], *z.shape[1:]), z.dtype) for z in zero_outs
    ]
    shard = jax.sharding.NamedSharding(mesh, PartitionSpec("core"))
    args = [jax.device_put(a, shard) for a in concat_in + concat_zeros]

    def run():
        o = fn(*args)[0]
        return jax.block_until_ready(o)

    run.fn = fn
    run.args = args
    return run, out_names, out_avals


def timed_run(x, w1, b1, w2, b2, gamma, r1=2, r2=10, reps=15,
              build=None):
    """Measure per-kernel device time via the slope between two NEFFs that
    run the whole kernel body `r1` and `r2` times internally (the constant
    axon RPC overhead cancels in the difference)."""
    import time as _time

    x = np.ascontiguousarray(np.asarray(x, dtype=np.float32))
    A = float(np.asarray(w1, np.float32) @ np.asarray(w2, np.float32))
    B2c = float(np.asarray(b1, np.float32) @ np.asarray(w2, np.float32))
    gam = float(np.asarray(gamma, np.float32).reshape(-1)[0])
    eye = np.eye(128, dtype=np.float32)
    xs = x[:, 0].reshape(N_CORES, SPC * C, T)
    in_maps = [{"x": np.ascontiguousarray(xs[r]), "ident": eye} for r in range(N_CORES)]

    t_best = {}
    out_arr = None
    out_avals = None
    if build is None:
        def build(A_, B2_, g_, replicate=1):
            return BUILD(A_, B2_, g_, replicate=replicate, **BUILD_KWARGS)
    for rep in (r1, r2):
        nc = build(A, B2c, gam, replicate=rep)
        run, out_names, out_avals = make_timed_runner(nc, in_maps)
        out_arr = run()  # compile + warmup
        run()
        best = None
        for _ in range(reps):
            t0 = _time.perf_counter_ns()
            run()
            dt = _time.perf_counter_ns() - t0
            best = dt if best is None else min(best, dt)
        t_best[rep] = best

    per_exec_ns = (t_best[r2] - t_best[r1]) / (r2 - r1)
    out = np.asarray(out_arr)
    out = out.reshape(N_CORES, *out_avals[0].shape).reshape(B, C, T)[:, None]
    return out.astype(np.float32), per_exec_ns


# revision 44
# speedup vs baseline: 2736.3462x; 1.3048x over previous
"""ChannelSelfAttn Trainium2 kernel.

Reference computation (per sample b, x_b: [C=64, T=4000]):
    q = w1*x + b1, k = w2*x + b2 broadcast over F=16 feature maps
    e[i,j] = sum_{f,t} q[f,i,t]*k[f,j,t]
           = A*G[i,j] + B1*s_i + B2*s_j + C0*T
      where G = x_b @ x_b.T, s = rowsum(x_b),
            A = w1.w2, B1 = w1.b2, B2 = b1.w2, C0 = b1.b2
    e <- (e - min_j e)/(max_j e - min_j e + 1e-8)   # row terms B1*s_i, C0*T cancel
    e <- softmax_j(e)
    out = gamma * (e @ x_b) + x_b

So only f = A*G + B2*s_j survives the normalize; everything reduces to a
64x64 gram matrix + row-softmax + a second small matmul per sample.

Sharding: data-parallel over batch. 32 samples / 8 cores = 4 samples/core,
processed as 2 pairs; each pair stacks 2 samples' channels into the 128
SBUF partitions.
"""

import numpy as np

import concourse.bacc as bacc
import concourse.bass as bass
import concourse.mybir as mybir
import concourse.tile as tile
from concourse.bass_utils import run_bass_kernel_spmd

FP32 = mybir.dt.float32
AF = mybir.ActivationFunctionType
ALU = mybir.AluOpType
AX = mybir.AxisListType

B, C, T = 32, 64, 4000
N_CORES = 8
SPC = B // N_CORES          # samples per core = 4
PAIRS = SPC // 2            # 2
TPAD = 4096                 # T padded to 32 chunks of 128
NCHUNK = TPAD // 128        # 32
NATT = TPAD // 512          # 8 attention N-chunks


def build_program(A: float, B2: float, gamma: float, replicate: int = 1,
                  f32r_attn: bool = False, f32r_trans: bool = False,
                  f32r_gram: bool = False) -> bass.Bass:
    FP32R = mybir.dt.float32r

    def ra(ap):  # attn operands
        return ap.bitcast(FP32R) if f32r_attn else ap

    def rt(ap):  # transpose operands
        return ap.bitcast(FP32R) if f32r_trans else ap

    def rg(ap):  # gram operands
        return ap.bitcast(FP32R) if f32r_gram else ap

    nc = bacc.Bacc(None)
    x_h = nc.declare_dram_parameter("x", [SPC * C, T], FP32, isOutput=False)
    id_h = nc.declare_dram_parameter("ident", [128, 128], FP32, isOutput=False)
    out_h = nc.declare_dram_parameter("out", [SPC * C, T], FP32, isOutput=True)

    with tile.TileContext(nc) as tc:
        with (
            tc.tile_pool(name="xio", bufs=2) as p_x,
            tc.tile_pool(name="xT", bufs=2) as p_xT,
            tc.tile_pool(name="outb", bufs=2) as p_out,
            tc.tile_pool(name="small", bufs=2) as p_small,
            tc.tile_pool(name="const", bufs=1) as p_const,
            tc.tile_pool(name="pt", bufs=2, space="PSUM") as p_pt,
            tc.tile_pool(name="pg", bufs=2, space="PSUM") as p_pg,
            tc.tile_pool(name="pa", bufs=2, space="PSUM") as p_pa,
            tc.tile_pool(name="ps", bufs=2, space="PSUM") as p_ps,
        ):
            ident = p_const.tile([128, 128], FP32)
            nc.sync.dma_start(ident[:], id_h[:, :])
            ones = p_const.tile([1, 128], FP32)
            nc.vector.memset(ones[:], 1.0)

            for p in [pp for _ in range(replicate) for pp in range(PAIRS)]:
                rows = slice(p * 128, (p + 1) * 128)

                # ---- load x pair [128, 4000], zero-pad t to 4096
                x_stack = p_x.tile([128, TPAD], FP32)
                for i in range(4):
                    c0, c1 = i * 1000, (i + 1) * 1000
                    nc.sync.dma_start(x_stack[:, c0:c1], x_h[rows, c0:c1])
                nc.vector.memset(x_stack[:, T:TPAD], 0.0)

                # ---- transpose to xT [t, c] chunks via PE
                xT = p_xT.tile([128, TPAD], FP32)
                for q in range(NCHUNK // 4):
                    pt = p_pt.tile([128, 512], FP32, tag="pt")
                    for j in range(4):
                        k = 4 * q + j
                        nc.tensor.transpose(
                            rt(pt[:, j * 128:(j + 1) * 128]),
                            rt(x_stack[:, k * 128:(k + 1) * 128]),
                            rt(ident[:, :]),
                        )
                    dst = xT[:, q * 512:(q + 1) * 512]
                    if q % 2 == 0:
                        nc.vector.tensor_copy(dst, pt[:])
                    else:
                        nc.scalar.copy(dst, pt[:])

                # ---- gram: pg = sum_k xT_k.T @ xT_k  -> [[G_A, .], [., G_B]]
                pg = p_pg.tile([128, 128], FP32, tag="pg")
                for k in range(NCHUNK):
                    blkk = rg(xT[:, k * 128:(k + 1) * 128])
                    nc.tensor.matmul(
                        pg[:], lhsT=blkk, rhs=blkk,
                        start=(k == 0), stop=(k == NCHUNK - 1),
                    )

                # ---- s = rowsum(x), then broadcast B2*s as a row
                s_col = p_small.tile([128, 1], FP32, tag="scol")
                nc.vector.reduce_sum(s_col[:], x_stack[:], axis=AX.X)
                ps_row = p_ps.tile([1, 128], FP32, tag="ps")
                nc.tensor.transpose(ps_row[:], s_col[:], ident[:, :])
                srow_b = p_small.tile([1, 128], FP32, tag="srowb")
                nc.scalar.mul(srow_b[:], ps_row[:], B2)
                # broadcast row down all partitions: psb[m, j] = B2*s_j
                psb = p_ps.tile([128, 128], FP32, tag="ps")
                nc.tensor.matmul(psb[:], lhsT=ones[:], rhs=srow_b[:], start=True, stop=True)
                sbs = p_small.tile([128, C], FP32, tag="sbs")
                nc.scalar.copy(sbs[0:64, :], psb[0:64, 0:64])
                nc.scalar.copy(sbs[64:128, :], psb[64:128, 64:128])

                # ---- f = A*G + B2*s_j (diag blocks only), stacked [128, 64]
                fs = p_small.tile([128, C], FP32, tag="fs")
                nc.vector.scalar_tensor_tensor(
                    fs[0:64, :], pg[0:64, 0:64], A, sbs[0:64, :],
                    op0=ALU.mult, op1=ALU.add,
                )
                nc.vector.scalar_tensor_tensor(
                    fs[64:128, :], pg[64:128, 64:128], A, sbs[64:128, :],
                    op0=ALU.mult, op1=ALU.add,
                )

                # ---- row minmax-normalize + exp (+ rowsum for softmax denom)
                mx = p_small.tile([128, 1], FP32, tag="mx")
                nc.vector.reduce_max(mx[:], fs[:], axis=AX.X)
                mn = p_small.tile([128, 1], FP32, tag="mn")
                nc.vector.tensor_reduce(mn[:], fs[:], axis=AX.X, op=ALU.min)
                dd = p_small.tile([128, 1], FP32, tag="dd")
                nc.vector.scalar_tensor_tensor(
                    dd[:], mx[:], 1e-8, mn[:], op0=ALU.add, op1=ALU.subtract,
                )
                rr = p_small.tile([128, 1], FP32, tag="rr")
                nc.vector.reciprocal(rr[:], dd[:])
                nb = p_small.tile([128, 1], FP32, tag="nb")
                nc.vector.scalar_tensor_tensor(
                    nb[:], mn[:], -1.0, rr[:], op0=ALU.mult, op1=ALU.mult,
                )
                # exp() written into diagonal blocks of a zeroed [128,128] tile
                # so ONE transpose yields the block-diag lhsT for the attn matmul
                gw = p_small.tile([128, 128], FP32, tag="gw")
                nc.vector.memset(gw[:], 0.0)
                se = p_small.tile([128, 1], FP32, tag="se")
                nc.scalar.activation(
                    gw[0:64, 0:64], fs[0:64, :], AF.Exp,
                    bias=nb[0:64], scale=rr[0:64], accum_out=se[0:64],
                )
                nc.scalar.activation(
                    gw[64:128, 64:128], fs[64:128, :], AF.Exp,
                    bias=nb[64:128], scale=rr[64:128], accum_out=se[64:128],
                )
                rs = p_small.tile([128, 1], FP32, tag="rs")
                nc.vector.reciprocal(rs[:], se[:])
                wsc = p_small.tile([128, 1], FP32, tag="wsc")
                nc.vector.tensor_scalar_mul(wsc[:], rs[:], gamma)

                # ---- block-diag transposed weights for attn matmul
                pb = p_ps.tile([128, 128], FP32, tag="ps")
                nc.tensor.transpose(pb[:], gw[:], ident[:, :])
                blk = p_small.tile([128, 128], FP32, tag="blk")
                nc.vector.tensor_copy(blk[:], pb[:])

                # ---- attn = blk.T @ x (unnormalized), out = wsc*attn + x
                ob = p_out.tile([128, TPAD], FP32)
                for n in range(NATT):
                    pa = p_pa.tile([128, 512], FP32, tag="pa")
                    xch = x_stack[:, n * 512:(n + 1) * 512]
                    nc.tensor.matmul(pa[:], lhsT=ra(blk[:]), rhs=ra(xch),
                                     start=True, stop=True)
                    nc.vector.scalar_tensor_tensor(
                        ob[:, n * 512:(n + 1) * 512], pa[:], wsc[:], xch,
                        op0=ALU.mult, op1=ALU.add,
                    )

                for i in range(4):
                    c0, c1 = i * 1000, (i + 1) * 1000
                    nc.sync.dma_start(out_h[rows, c0:c1], ob[:, c0:c1])

    nc.finalize()
    return nc


def build_program_f32r(A: float, B2: float, gamma: float, replicate: int = 1,
                       fuse_s: bool = True, dma_pieces: int = 4,
                       xr_gp: bool = False, pa3: bool = False,
                       exact_g: bool = False, ring_split: bool = False,
                       pt3: bool = False, ob_split: bool = False,
                       out_pieces: int = 0) -> bass.Bass:
    """float32r variant: matmul/transpose operands in fp32r (reduced-precision
    fp32 that streams at full PE rate for moving>=256). All f32r operands are
    produced by compute ops (rounding); the +x residual stays exact fp32.
    Gram uses a 256-wide moving window (right half is discarded garbage) to
    hit the f32r full-rate threshold. With fuse_s, xT chunks are laid out at
    stride 129 with a ones column appended, so PSUM column 128 of the gram
    accumulates the row-sum s for free (no big DVE reduction)."""
    FP32R = mybir.dt.float32r
    GDT = FP32 if exact_g else FP32R     # dtype of transpose/gram pipeline
    GW_RHS = (129 if exact_g else 256) if fuse_s else (128 if exact_g else 256)
    CW = 129 if fuse_s else 128          # xT column stride per chunk
    XTW = NCHUNK * CW + max(GW_RHS - CW, 0)

    nc = bacc.Bacc(None)
    x_h = nc.declare_dram_parameter("x", [SPC * C, T], FP32, isOutput=False)
    id_h = nc.declare_dram_parameter("ident", [128, 128], FP32, isOutput=False)
    out_h = nc.declare_dram_parameter("out", [SPC * C, T], FP32, isOutput=True)

    with tile.TileContext(nc) as tc:
        with (
            tc.tile_pool(name="xio", bufs=2) as p_x,
            tc.tile_pool(name="xr", bufs=2) as p_xr,
            tc.tile_pool(name="xT", bufs=2) as p_xT,
            tc.tile_pool(name="outb", bufs=2) as p_out,
            tc.tile_pool(name="small", bufs=2) as p_small,
            tc.tile_pool(name="const", bufs=1) as p_const,
            tc.tile_pool(name="pt", bufs=3 if pt3 else 2, space="PSUM") as p_pt,
            tc.tile_pool(name="pg", bufs=2, space="PSUM") as p_pg,
            tc.tile_pool(name="pa", bufs=3 if pa3 else 2, space="PSUM") as p_pa,
            tc.tile_pool(name="ps", bufs=1 if (pa3 or pt3) else 2, space="PSUM") as p_ps,
        ):
            ident = p_const.tile([128, 128], FP32)
            nc.sync.dma_start(ident[:], id_h[:, :])
            identr = p_const.tile([128, 128], FP32R)
            nc.vector.tensor_copy(identr[:], ident[:])
            ones_f = p_const.tile([1, 128], FP32)
            nc.vector.memset(ones_f[:], 1.0)
            ones = p_const.tile([1, 128], FP32R)
            nc.vector.tensor_copy(ones[:], ones_f[:])
            zsrc = p_const.tile([128, 128], FP32)
            nc.vector.memset(zsrc[:], 0.0)
            c_ones = p_const.tile([128, NCHUNK], FP32)
            nc.vector.memset(c_ones[:], 1.0)

            for p in [pp for _ in range(replicate) for pp in range(PAIRS)]:
                rows = slice(p * 128, (p + 1) * 128)

                # ---- load x pair [128, 4000] fp32, zero-pad t to 4096
                x_stack = p_x.tile([128, TPAD], FP32)
                npc = T // dma_pieces
                for i in range(dma_pieces):
                    c0, c1 = i * npc, (i + 1) * npc
                    eng = nc.scalar if (ring_split and i % 2) else nc.sync
                    eng.dma_start(x_stack[:, c0:c1], x_h[rows, c0:c1])
                nc.vector.memset(x_stack[:, T:TPAD], 0.0)

                # ---- rounded fp32r copy (feeds every matmul operand)
                xr = p_xr.tile([128, TPAD], FP32R)
                if xr_gp:
                    nc.gpsimd.tensor_copy(xr[:, 0:2048], x_stack[:, 0:2048])
                    nc.gpsimd.tensor_copy(xr[:, 2048:TPAD], x_stack[:, 2048:TPAD])
                else:
                    nc.vector.tensor_copy(xr[:, 0:2048], x_stack[:, 0:2048])
                    nc.scalar.copy(xr[:, 2048:TPAD], x_stack[:, 2048:TPAD])

                # ---- transpose to xT [t, c] chunks via PE
                xT = p_xT.tile([128, XTW], GDT)
                if XTW > NCHUNK * CW:
                    nc.vector.tensor_copy(xT[:, NCHUNK * CW:XTW], zsrc[:, 0:XTW - NCHUNK * CW])
                if fuse_s:
                    # ones column after each chunk's 128 data columns
                    oview = xT[:, 0:NCHUNK * CW].rearrange("p (k c) -> p k c", c=CW)
                    nc.vector.tensor_copy(oview[:, :, 128:129], c_ones[:].rearrange("p (k o) -> p k o", o=1))
                tsrc = x_stack if exact_g else xr
                tid = ident if exact_g else identr
                for q in range(NCHUNK // 4):
                    pt = p_pt.tile([128, 512], GDT, tag="pt")
                    for j in range(4):
                        k = 4 * q + j
                        nc.tensor.transpose(
                            pt[:, j * 128:(j + 1) * 128],
                            tsrc[:, k * 128:(k + 1) * 128],
                            tid[:, :],
                        )
                    if fuse_s:
                        dst = xT[:, q * 4 * CW:(q * 4 + 4) * CW].rearrange(
                            "p (k c) -> p k c", c=CW)[:, :, 0:128]
                        src = pt[:].rearrange("p (k c) -> p k c", c=128)
                    else:
                        dst = xT[:, q * 512:(q + 1) * 512]
                        src = pt[:]
                    if q % 2 == 0:
                        nc.vector.tensor_copy(dst, src)
                    else:
                        nc.scalar.copy(dst, src)

                # ---- gram, 256-wide moving (full f32r rate); left half valid.
                # With fuse_s, psum col 128 accumulates s = rowsum(x) as well.
                pg = p_pg.tile([128, GW_RHS], FP32, tag="pg")
                for k in range(NCHUNK):
                    nc.tensor.matmul(
                        pg[:], lhsT=xT[:, k * CW:k * CW + 128],
                        rhs=xT[:, k * CW:k * CW + GW_RHS],
                        start=(k == 0), stop=(k == NCHUNK - 1),
                    )

                # ---- B2*s as a row via transpose + rank-1 broadcast matmul
                s_colr = p_small.tile([128, 1], GDT, tag="scolr")
                if fuse_s:
                    nc.scalar.mul(s_colr[:], pg[:, 128:129], B2)
                else:
                    s_col = p_small.tile([128, 1], FP32, tag="scol")
                    nc.vector.reduce_sum(s_col[:], x_stack[:], axis=AX.X)
                    nc.scalar.mul(s_colr[:], s_col[:], B2)
                ps_row = p_ps.tile([1, 128], GDT, tag="ps")
                nc.tensor.transpose(ps_row[:], s_colr[:], tid[:, :])
                srow_b = p_small.tile([1, 128], GDT, tag="srowb")
                nc.vector.tensor_copy(srow_b[:], ps_row[:])
                psb = p_ps.tile([128, 128], FP32, tag="ps")
                nc.tensor.matmul(psb[:], lhsT=(ones_f if exact_g else ones)[:],
                                 rhs=srow_b[:], start=True, stop=True)
                sbs = p_small.tile([128, C], FP32, tag="sbs")
                nc.scalar.copy(sbs[0:64, :], psb[0:64, 0:64])
                nc.scalar.copy(sbs[64:128, :], psb[64:128, 64:128])

                # ---- f = A*G + B2*s_j (diag blocks only), stacked [128, 64]
                fs = p_small.tile([128, C], FP32, tag="fs")
                nc.vector.scalar_tensor_tensor(
                    fs[0:64, :], pg[0:64, 0:64], A, sbs[0:64, :],
                    op0=ALU.mult, op1=ALU.add,
                )
                nc.vector.scalar_tensor_tensor(
                    fs[64:128, :], pg[64:128, 64:128], A, sbs[64:128, :],
                    op0=ALU.mult, op1=ALU.add,
                )

                # ---- row minmax-normalize + exp (+ rowsum for softmax denom)
                mx = p_small.tile([128, 1], FP32, tag="mx")
                nc.vector.reduce_max(mx[:], fs[:], axis=AX.X)
                mn = p_small.tile([128, 1], FP32, tag="mn")
                nc.vector.tensor_reduce(mn[:], fs[:], axis=AX.X, op=ALU.min)
                dd = p_small.tile([128, 1], FP32, tag="dd")
                nc.vector.scalar_tensor_tensor(
                    dd[:], mx[:], 1e-8, mn[:], op0=ALU.add, op1=ALU.subtract,
                )
                rr = p_small.tile([128, 1], FP32, tag="rr")
                nc.vector.reciprocal(rr[:], dd[:])
                nb = p_small.tile([128, 1], FP32, tag="nb")
                nc.vector.scalar_tensor_tensor(
                    nb[:], mn[:], -1.0, rr[:], op0=ALU.mult, op1=ALU.mult,
                )
                gw = p_small.tile([128, 128], FP32R, tag="gw")
                nc.vector.tensor_copy(gw[:], zsrc[:])
                se = p_small.tile([128, 1], FP32, tag="se")
                nc.scalar.activation(
                    gw[0:64, 0:64], fs[0:64, :], AF.Exp,
                    bias=nb[0:64], scale=rr[0:64], accum_out=se[0:64],
                )
                nc.scalar.activation(
                    gw[64:128, 64:128], fs[64:128, :], AF.Exp,
                    bias=nb[64:128], scale=rr[64:128], accum_out=se[64:128],
                )
                rs = p_small.tile([128, 1], FP32, tag="rs")
                nc.vector.reciprocal(rs[:], se[:])
                wsc = p_small.tile([128, 1], FP32, tag="wsc")
                nc.vector.tensor_scalar_mul(wsc[:], rs[:], gamma)

                # ---- block-diag transposed weights for attn matmul
                pb = p_ps.tile([128, 128], FP32R, tag="ps")
                nc.tensor.transpose(pb[:], gw[:], identr[:, :])
                blk = p_small.tile([128, 128], FP32R, tag="blk")
                nc.vector.tensor_copy(blk[:], pb[:])

                # ---- attn = blk.T @ xr (unnormalized), out = wsc*attn + x
                ob = p_out.tile([128, TPAD], FP32)
                for n in range(NATT):
                    pa = p_pa.tile([128, 512], FP32, tag="pa")
                    nc.tensor.matmul(pa[:], lhsT=blk[:], rhs=xr[:, n * 512:(n + 1) * 512],
                                     start=True, stop=True)
                    obc = ob[:, n * 512:(n + 1) * 512]
                    xc = x_stack[:, n * 512:(n + 1) * 512]
                    if ob_split and n % 2 == 1:
                        # rebalance: scale on ACT, residual add on DVE
                        nc.scalar.mul(obc, pa[:], wsc[:])
                        nc.vector.tensor_add(obc, obc, xc)
                    else:
                        nc.vector.scalar_tensor_tensor(
                            obc, pa[:], wsc[:], xc, op0=ALU.mult, op1=ALU.add,
                        )

                nop = out_pieces or dma_pieces
                opc = T // nop
                for i in range(nop):
                    c0, c1 = i * opc, (i + 1) * opc
                    eng = nc.scalar if (ring_split and i % 2 == 0) else nc.sync
                    eng.dma_start(out_h[rows, c0:c1], ob[:, c0:c1])

    nc.finalize()
    return nc


# Final kernel configuration (selected by on-hardware benchmarking)
BUILD = build_program_f32r
BUILD_KWARGS = {"fuse_s": True, "ob_split": True, "out_pieces": 8}


def _run(x, w1, b1, w2, b2, gamma, **run_kwargs):
    x = np.ascontiguousarray(np.asarray(x, dtype=np.float32))
    w1 = np.asarray(w1, dtype=np.float32)
    b1 = np.asarray(b1, dtype=np.float32)
    w2 = np.asarray(w2, dtype=np.float32)
    b2 = np.asarray(b2, dtype=np.float32)
    gamma = np.asarray(gamma, dtype=np.float32)
    assert x.shape == (B, 1, C, T), x.shape

    A = float(w1 @ w2)
    B2c = float(b1 @ w2)
    gam = float(gamma.reshape(-1)[0])

    nc = BUILD(A, B2c, gam, **BUILD_KWARGS)

    eye = np.eye(128, dtype=np.float32)
    xs = x[:, 0].reshape(N_CORES, SPC * C, T)
    in_maps = [{"x": np.ascontiguousarray(xs[r]), "ident": eye} for r in range(N_CORES)]
    res = run_bass_kernel_spmd(nc, in_maps, list(range(N_CORES)), **run_kwargs)
    out = np.stack([res.results[r]["out"] for r in range(N_CORES)])
    out = out.reshape(B, C, T)[:, None].astype(np.float32)
    return out, res


def kernel(x, w1, b1, w2, b2, gamma):
    out, _ = _run(x, w1, b1, w2, b2, gamma)
    return out


def make_timed_runner(nc, in_maps):
    """Build a jitted 8-core runner (no donation) for repeat timing.

    Mirrors bass2jax.run_bass_via_pjrt's multi-core path but keeps the jitted
    function so the NEFF can be executed repeatedly with device-resident args.
    """
    import jax
    import numpy as _np
    from jax.sharding import Mesh, PartitionSpec
    from jax.experimental.shard_map import shard_map

    import concourse.mybir as _mybir
    from concourse import bass2jax
    from concourse.bass2jax import _bass_exec_p, install_neuronx_cc_hook

    install_neuronx_cc_hook()
    n_cores = len(in_maps)
    partition_name = nc.partition_id_tensor.name if nc.partition_id_tensor else None

    in_names, out_names, out_avals, zero_outs = [], [], [], []
    for alloc in nc.m.functions[0].allocations:
        if not isinstance(alloc, _mybir.MemoryLocationSet):
            continue
        name = alloc.memorylocations[0].name
        if alloc.kind == "ExternalInput":
            if name != partition_name:
                in_names.append(name)
        elif alloc.kind == "ExternalOutput":
            out_names.append(name)
            shape = tuple(alloc.tensor_shape)
            dtype = _mybir.dt.np(alloc.dtype)
            out_avals.append(jax.core.ShapedArray(shape, dtype))
            zero_outs.append(_np.zeros(shape, dtype))
    n_params = len(in_names)
    in_names = in_names + out_names
    if partition_name is not None:
        in_names.append(partition_name)

    def _exec_once(*args):
        operands = list(args)
        if partition_name is not None:
            operands.append(bass2jax.partition_id_tensor())
        outs = _bass_exec_p.bind(
            *operands,
            out_avals=tuple(out_avals),
            in_names=tuple(in_names),
            out_names=tuple(out_names),
            lowering_input_output_aliases=(),
            sim_require_finite=True,
            sim_require_nnan=True,
            nc=nc,
        )
        return tuple(outs)

    assert len(out_names) == 1

    devices = jax.devices()[:n_cores]
    mesh = Mesh(_np.asarray(devices), ("core",))
    in_specs = (PartitionSpec("core"),) * (n_params + len(out_names))
    out_specs = (PartitionSpec("core"),) * len(out_names)
    fn = jax.jit(
        shard_map(_exec_once, mesh=mesh, in_specs=in_specs, out_specs=out_specs,
                  check_rep=False),
        keep_unused=True,
    )
    concat_in = [
        _np.concatenate([_np.asarray(in_maps[c][nm]) for c in range(n_cores)], axis=0)
        for nm in in_names[:n_params]
    ]
    concat_zeros = [
        _np.zeros((n_cores * z.shape[0], *z.shape[1:]), z.dtype) for z in zero_outs
    ]
    shard = jax.sharding.NamedSharding(mesh, PartitionSpec("core"))
    args = [jax.device_put(a, shard) for a in concat_in + concat_zeros]

    def run():
        o = fn(*args)[0]
        return jax.block_until_ready(o)

    run.fn = fn
    run.args = args
    return run, out_names, out_avals


def timed_run(x, w1, b1, w2, b2, gamma, r1=2, r2=10, reps=15,
              build=None):
    """Measure per-kernel device time via the slope between two NEFFs that
    run the whole kernel body `r1` and `r2` times internally (the constant
    axon RPC overhead cancels in the difference)."""
    import time as _time

    x = np.ascontiguousarray(np.asarray(x, dtype=np.float32))
    A = float(np.asarray(w1, np.float32) @ np.asarray(w2, np.float32))
    B2c = float(np.asarray(b1, np.float32) @ np.asarray(w2, np.float32))
    gam = float(np.asarray(gamma, np.float32).reshape(-1)[0])
    eye = np.eye(128, dtype=np.float32)
    xs = x[:, 0].reshape(N_CORES, SPC * C, T)
    in_maps = [{"x": np.ascontiguousarray(xs[r]), "ident": eye} for r in range(N_CORES)]

    t_best = {}
    out_arr = None
    out_avals = None
    if build is None:
        def build(A_, B2_, g_, replicate=1):
            return BUILD(A_, B2_, g_, replicate=replicate, **BUILD_KWARGS)
    for rep in (r1, r2):
        nc = build(A, B2c, gam, replicate=rep)
        run, out_names, out_avals = make_timed_runner(nc, in_maps)
        out_arr = run()  # compile + warmup
        run()
        best = None
        for _ in range(reps):
            t0 = _time.perf_counter_ns()
            run()
            dt = _time.perf_counter_ns() - t0
            best = dt if best is None else min(best, dt)
        t_best[rep] = best

    per_exec_ns = (t_best[r2] - t_best[r1]) / (r2 - r1)
    out = np.asarray(out_arr)
    out = out.reshape(N_CORES, *out_avals[0].shape).reshape(B, C, T)[:, None]
    return out.astype(np.float32), per_exec_ns
